# revision 7
# baseline (speedup 1.0000x reference)
"""BertAttention (B=2,S=2048,D=1024,H=16) on 8 trn2 NeuronCores.

Sharding: data-parallel over B (2 groups of 4 cores); each group's 4 cores
split the 2048 query rows (512 each). Every core computes K^T and V for its
batch in full (redundant within the group; collectives in this runtime cost
15us+ fixed which is worse than the ~40us of redundant PE), its own 512-row
Q slice, attention over all 16 heads, output projection, residual, LayerNorm.
Key columns are host-rotated per core so columns 0:512 of x^T are always the
core's own query block (softmax is key-order invariant) — every core runs an
identical schedule.

v2 structure (one-pass waves, JIT production):
  - All projections run fp8e4 DoubleRow (weights host-scaled by 64; 1/64
    descale folded into the PSUM drains). Biases eliminated exactly: bk
    drops (softmax shift-invariance), bv/bo fold into the host residual,
    bq rides the Q^T drain.
  - Attention is 8 waves x 8 steps; wave w = head pair (2w, 2w+1), step p
    = key tiles 2p,2p+1. The two heads' scores matmuls contract only 64
    partitions and sit on partitions 0:64 / 64:128, so they are issued
    back-to-back and execute CONCURRENTLY in the PE array via row tiling
    (tile_position (0,0) vs (64,0)) — 2x scores throughput.
  - exp is one fused ACT op per (step, head) (scale=1/8, max-free: scores/8
    is in [-3.6,3.6] here), writing fp8 directly. The mask enters as
    exp(mask)/64 folded into V's rows, with exp(mask)/64 in V's 65th column
    so PV row 64 is the softmax denominator /64.
  - PV (fp8 DR) is lagged one step behind scores/exp so the PE never waits
    on the ACT engine; each head's PV accumulates all 16 key tiles in one
    PSUM bank (no SBUF accumulators, no halves).
  - Softmax denominators: reciprocal on DVE, then partition-broadcast via a
    tiny ones-stationary bf16 matmul on the PE (no DRAM bounce).
  - Q/K/V production units are drip-fed into the wave stream on a static
    deadline schedule (emitted >=1 step before first use; PE executes them
    in the slack while ACT runs exp).
  - Tail: all four query-block output projections use all 8 PSUM banks,
    drain+LayerNorm pipelined per block (beta-add on gpsimd for blocks 0-2
    so the DVE chain stays short).
"""

import numpy as np

B, S, D, H = 2, 2048, 1024, 16
HD = D // H  # 64
HD1 = HD + 1
P = 128
NCORES = 8
SQ = S // 4  # 512 query rows per core
DT = D // P  # 8 feature tiles
KS = S // P  # 16 key tiles
WS = 64.0  # host-side weight scale for fp8
EPS = 1e-12
CW = DT + KS + 2 * D  # consts blob width

_CACHE = {}


def _ensure_paths():
    try:
        import concourse  # noqa: F401
    except ImportError:
        import sys

        for p in ("/opt/trn_rl_repo", "/root/.axon_site/_ro/trn_rl_repo"):
            if p not in sys.path:
                sys.path.append(p)
        import concourse  # noqa: F401


def _drip_schedule():
    """Static emission plan for production units.

    Unit kinds: ("q", dt, None), ("k", dt, kc), ("v", kt, nd).
    deadline = first attention step (w*8+p) whose PE instructions read the
    unit's output. Units are levelled into earlier surplus steps, never
    later than deadline-1 (PE executes in issue order; a unit issued at or
    after its consumer would deadlock the in-order queue).
    """
    units = []
    for dt in range(DT):
        units.append(("q", dt, None))
        for kc in range(4):
            units.append(("k", dt, kc))
    for kt in range(KS):
        for nd in range(2):
            units.append(("v", kt, nd))

    def deadline(u):
        kind, a, b = u
        if kind == "q":
            return a * 8
        if kind == "k":
            return a * 8 + 2 * b
        return b * 32 + a // 2  # v: PV flush lags 1 step anyway

    prefix = [("q", 0, None), ("k", 0, 0), ("v", 0, 0), ("v", 1, 0)]
    rest = [u for u in units if u not in prefix]
    rest.sort(key=deadline)
    plan = [[] for _ in range(64)]
    load = [0] * 64
    for u in rest:
        dl = max(0, deadline(u) - 1)
        s = 0
        while s < dl and load[s] >= 2:
            s += 1
        plan[s].append(u)
        load[s] += 1
    return prefix, plan


def build_nc():
    _ensure_paths()
    import concourse.tile as tile
    from concourse import bacc, mybir

    f32 = mybir.dt.float32
    bf16 = mybir.dt.bfloat16
    f8 = mybir.dt.float8e4
    DR = mybir.MatmulPerfMode.DoubleRow
    AF = mybir.ActivationFunctionType
    OP = mybir.AluOpType

    nc = bacc.Bacc()

    # ---- I/O ----
    xT8 = nc.declare_dram_parameter("xT8", [D, S], f8, isOutput=False)
    xq = nc.declare_dram_parameter("xq", [SQ, D], f32, isOutput=False)
    Wq = nc.declare_dram_parameter("Wq8", [D, D], f8, isOutput=False)
    Wk = nc.declare_dram_parameter("Wk8", [D, D], f8, isOutput=False)
    Wv = nc.declare_dram_parameter("Wv8", [D, D], f8, isOutput=False)
    Wo = nc.declare_dram_parameter("Wo8", [D, D], f8, isOutput=False)
    # consts blob: [bq_t | emask_t | gamma_bc | beta_bc]
    cst = nc.declare_dram_parameter("cst", [P, CW], f32, isOutput=False)
    out = nc.declare_dram_parameter("out", [SQ, D], f32, isOutput=True)

    xT_r = xT8.rearrange("(t p) s -> p t s", p=P)  # [128, 8, 2048]
    xq_r = xq.rearrange("(t p) d -> p t d", p=P)  # [128, 4, 1024]
    W_r = {
        "q": Wq.rearrange("(t p) d -> p t d", p=P),
        "k": Wk.rearrange("(t p) d -> p t d", p=P),
        "v": Wv.rearrange("(t p) d -> p t d", p=P),
        "o": Wo.rearrange("(t p) d -> p t d", p=P),
    }
    out_r = out.rearrange("(t p) d -> t p d", p=P)  # [4, 128, 1024]

    def mm(ps, lhsT, rhs, start, stop, perf_mode=None):
        nc.tensor.matmul(ps, lhsT, rhs, start=start, stop=stop, perf_mode=perf_mode)

    prefix, plan = _drip_schedule()

    with tile.TileContext(nc) as tc:
        with (
            tc.tile_pool(name="consts", bufs=1) as consts,
            tc.tile_pool(name="pers", bufs=1) as pers,
            tc.tile_pool(name="wpool", bufs=1) as wpool,
            tc.tile_pool(name="expt", bufs=6) as ex_pool,
            tc.tile_pool(name="sums", bufs=2) as sums_pool,
            tc.tile_pool(name="xbuf", bufs=4) as xb_pool,
            tc.tile_pool(name="stats", bufs=4) as st_pool,
            tc.tile_pool(name="ps_sc", bufs=2, space="PSUM") as ps_sc,
            tc.tile_pool(name="ps_pv", bufs=2, space="PSUM") as ps_pv,
            tc.tile_pool(name="ps_a", bufs=2, space="PSUM") as ps_a,
        ):
            # ---- persistent SBUF ----
            qt_sb = pers.tile([P, DT, SQ], f8)  # Q^T true scale
            kt_sb = pers.tile([P, DT, S], f8)  # K^T true scale
            v_sb = pers.tile([P, KS, H, HD1], f8)  # V*em rows + denom col
            ctxn = pers.tile([P, DT, SQ], f8)  # normalized ctx^T
            cst_sb = consts.tile([P, CW], f32)
            ones_sb = consts.tile([P, HD], bf16)
            eps_sb = consts.tile([P, 1], f32)
            wq_sb = wpool.tile([P, DT, D], f8, tag="Wq")
            wk_sb = wpool.tile([P, DT, D], f8, tag="Wk")
            wv_sb = wpool.tile([P, DT, D], f8, tag="Wv")
            wo_sb = wpool.tile([P, DT, D], f8, tag="Wo")
            xt8 = pers.tile([P, DT, S], f8)
            xq_sb = pers.tile([P, 4, D], f32)

            bq_sl = cst_sb[:, 0:DT]
            em_sl = cst_sb[:, DT : DT + KS]
            g_sl = cst_sb[:, DT + KS : DT + KS + D]
            be_sl = cst_sb[:, DT + KS + D : DT + KS + 2 * D]

            # ---- DMA wave-up (spread across queues; first-use order) ----
            nc.sync.dma_start(xt8[:, :, 0:SQ], xT_r[:, :, 0:SQ])
            nc.gpsimd.dma_start(cst_sb[:, 0 : DT + KS], cst[:, 0 : DT + KS])
            nc.scalar.dma_start(wq_sb[:, :, 0:P], W_r["q"][:, :, 0:P])
            nc.scalar.dma_start(wk_sb[:, :, 0:P], W_r["k"][:, :, 0:P])
            nc.scalar.dma_start(wv_sb[:, :, 0:SQ], W_r["v"][:, :, 0:SQ])
            nc.sync.dma_start(xt8[:, :, SQ : 2 * SQ], xT_r[:, :, SQ : 2 * SQ])
            nc.gpsimd.dma_start(wk_sb[:, :, P:D], W_r["k"][:, :, P:D])
            nc.sync.dma_start(xt8[:, :, 2 * SQ : 3 * SQ], xT_r[:, :, 2 * SQ : 3 * SQ])
            nc.sync.dma_start(xt8[:, :, 3 * SQ : 4 * SQ], xT_r[:, :, 3 * SQ : 4 * SQ])
            nc.gpsimd.dma_start(wq_sb[:, :, P:D], W_r["q"][:, :, P:D])
            nc.gpsimd.dma_start(wv_sb[:, :, SQ:D], W_r["v"][:, :, SQ:D])
            nc.gpsimd.dma_start(xq_sb[:], xq_r[:])
            nc.gpsimd.dma_start(wo_sb[:], W_r["o"][:])
            nc.gpsimd.dma_start(
                cst_sb[:, DT + KS :], cst[:, DT + KS :]
            )
            nc.vector.memset(ones_sb[:], 1.0)
            nc.vector.memset(eps_sb[:], EPS)

            # ---- production units ----
            def emit_q(dt):
                ps = ps_a.tile([P, SQ], f32, tag="kv", name=f"qu{dt}")
                for j in range(DT // 2):
                    mm(
                        ps[:],
                        wq_sb[:, 2 * j : 2 * j + 2, dt * P : (dt + 1) * P],
                        xt8[:, 2 * j : 2 * j + 2, 0:SQ],
                        start=(j == 0),
                        stop=(j == DT // 2 - 1),
                        perf_mode=DR,
                    )
                nc.vector.tensor_scalar(
                    out=qt_sb[:, dt, :],
                    in0=ps[:],
                    scalar1=1.0 / WS,
                    scalar2=bq_sl[:, dt : dt + 1],
                    op0=OP.mult,
                    op1=OP.add,
                )

            def emit_k(dt, kc):
                sl = slice(kc * SQ, (kc + 1) * SQ)
                ps = ps_a.tile([P, SQ], f32, tag="kv", name=f"ku{dt}_{kc}")
                for j in range(DT // 2):
                    mm(
                        ps[:],
                        wk_sb[:, 2 * j : 2 * j + 2, dt * P : (dt + 1) * P],
                        xt8[:, 2 * j : 2 * j + 2, sl],
                        start=(j == 0),
                        stop=(j == DT // 2 - 1),
                        perf_mode=DR,
                    )
                nc.vector.tensor_scalar_mul(
                    kt_sb[:, dt, sl], in0=ps[:], scalar1=1.0 / WS
                )

            def emit_v(kt, nd):
                ps = ps_a.tile([P, SQ], f32, tag="kv", name=f"vu{kt}_{nd}")
                for j in range(DT // 2):
                    mm(
                        ps[:],
                        xt8[:, 2 * j : 2 * j + 2, kt * P : (kt + 1) * P],
                        wv_sb[:, 2 * j : 2 * j + 2, nd * SQ : (nd + 1) * SQ],
                        start=(j == 0),
                        stop=(j == DT // 2 - 1),
                        perf_mode=DR,
                    )
                nc.vector.tensor_scalar_mul(
                    v_sb[:, kt, nd * 8 : (nd + 1) * 8, 0:HD],
                    in0=ps[:].rearrange("p (h c) -> p h c", c=HD),
                    scalar1=em_sl[:, kt : kt + 1],
                )
                if nd == 0:
                    # denominator column: exp(mask)/64 per key row. Must land
                    # with the nd=0 unit — wave-0 PV already reads col 64.
                    nc.vector.tensor_copy(
                        v_sb[:, kt, :, HD:HD1],
                        em_sl[:, kt : kt + 1].to_broadcast((P, H, 1)),
                    )

            def emit_unit(u):
                kind, a, b = u
                if kind == "q":
                    emit_q(a)
                elif kind == "k":
                    emit_k(a, b)
                else:
                    emit_v(a, b)

            for u in prefix:
                emit_unit(u)

            # ---- attention: 8 waves x 8 steps, PV lagged one step ----
            pend = {"pv": None}

            def normalize(w, pvts):
                bc = ps_sc.tile([P, 2, SQ], f32, tag="sc", name=f"bc{w}")
                for hh in range(2):
                    sf = sums_pool.tile(
                        [1, SQ], f32, tag=f"sf{hh}", name=f"sf{w}_{hh}"
                    )
                    nc.vector.tensor_copy(sf[:], pvts[hh][HD:HD1, :])
                    nc.vector.reciprocal_approx_fast(sf[:], sf[:])
                    sb16 = sums_pool.tile(
                        [1, SQ], bf16, tag=f"sb{hh}", name=f"sb{w}_{hh}"
                    )
                    nc.vector.tensor_copy(sb16[:], sf[:])
                    nc.tensor.matmul(
                        bc[0:HD, hh, :],
                        ones_sb[0:1, 0:HD],
                        sb16[:],
                        start=True,
                        stop=True,
                    )
                for hh in range(2):
                    off = hh * HD
                    bcs = sums_pool.tile(
                        [HD, SQ], f32, tag=f"bcs{hh}", name=f"bcs{w}_{hh}"
                    )
                    nc.vector.tensor_copy(bcs[:], bc[0:HD, hh, :])
                    nc.vector.scalar_tensor_tensor(
                        out=ctxn[off : off + HD, w, :],
                        in0=pvts[hh][0:HD, :],
                        scalar=1.0 / WS,
                        in1=bcs[:],
                        op0=OP.mult,
                        op1=OP.mult,
                    )

            def flush_pv():
                if pend["pv"] is None:
                    return
                w, p, pvts, exs = pend["pv"]
                pend["pv"] = None
                for hh in range(2):
                    h = 2 * w + hh
                    mm(
                        pvts[hh][:],
                        v_sb[:, 2 * p : 2 * p + 2, h, :],
                        exs[hh][:],
                        start=(p == 0),
                        stop=(p == KS // 2 - 1),
                        perf_mode=DR,
                    )
                if p == KS // 2 - 1:
                    normalize(w, pvts)

            for w in range(8):
                pvps = [
                    ps_pv.tile([HD1, SQ], f32, tag="pv", name=f"pv{w}_{hh}")
                    for hh in range(2)
                ]
                for p in range(8):
                    s = w * 8 + p
                    scs = [
                        ps_sc.tile([P, 2, SQ], f32, tag="sc", name=f"sc{s}_{hh}")
                        for hh in range(2)
                    ]
                    for u in range(2):
                        kt = 2 * p + u
                        for hh in range(2):
                            off = hh * HD
                            mm(
                                scs[hh][:, u, :],
                                kt_sb[off : off + HD, w, kt * P : (kt + 1) * P],
                                qt_sb[off : off + HD, w, :],
                                start=True,
                                stop=True,
                            )
                    flush_pv()
                    exs = []
                    for hh in range(2):
                        ex = ex_pool.tile(
                            [P, 2, SQ], f8, tag="ex", name=f"ex{s}_{hh}"
                        )
                        nc.scalar.activation(
                            ex[:], scs[hh][:], AF.Exp, scale=0.125
                        )
                        exs.append(ex)
                    pend["pv"] = (w, p, pvps, exs)
                    for u2 in plan[s]:
                        emit_unit(u2)

            flush_pv()  # last PV + normalize(7)

            # ---- tail: O proj (all 8 banks) + residual + LayerNorm ----
            def ln_qp(qp, xbuf):
                stats = st_pool.tile([P, 2, 6], f32, tag="st", name=f"st{qp}")
                xv = xbuf[:].rearrange("p (a d) -> p a d", a=2)
                for a in range(2):
                    nc.vector.bn_stats(stats[:, a, :], xv[:, a, :])
                mv = st_pool.tile([P, 2], f32, tag="mv", name=f"mv{qp}")
                nc.vector.bn_aggr(mv[:], stats[:])
                rstd = st_pool.tile([P, 1], f32, tag="rs", name=f"rs{qp}")
                nc.scalar.activation(rstd[:], mv[:, 1:2], AF.Sqrt, bias=eps_sb[:])
                nc.vector.reciprocal(rstd[:], rstd[:])
                nc.vector.tensor_scalar(
                    out=xbuf[:],
                    in0=xbuf[:],
                    scalar1=mv[:, 0:1],
                    scalar2=rstd[:],
                    op0=OP.subtract,
                    op1=OP.mult,
                )
                nc.vector.tensor_mul(xbuf[:], xbuf[:], g_sl[:])
                eng = nc.gpsimd if qp < 3 else nc.vector
                eng.tensor_add(xbuf[:], xbuf[:], be_sl[:])
                nc.sync.dma_start(out_r[qp], xbuf[:])

            for qp in range(4):
                if qp == 0:
                    pair = [
                        ps_a.tile([P, SQ], f32, tag="kv", name=f"oj0_{nd}")
                        for nd in range(2)
                    ]
                elif qp in (1, 2):
                    t = ps_sc.tile([P, 2, SQ], f32, tag="sc", name=f"oj{qp}")
                    pair = [t[:, 0, :], t[:, 1, :]]
                else:
                    pair = [
                        ps_pv.tile([P, SQ], f32, tag="pv", name=f"oj3_{nd}")
                        for nd in range(2)
                    ]
                for nd in range(2):
                    for p4 in range(4):
                        mm(
                            pair[nd],
                            ctxn[:, 2 * p4 : 2 * p4 + 2, qp * P : (qp + 1) * P],
                            wo_sb[:, 2 * p4 : 2 * p4 + 2, nd * SQ : (nd + 1) * SQ],
                            start=(p4 == 0),
                            stop=(p4 == 3),
                            perf_mode=DR,
                        )
                xbuf = xb_pool.tile([P, D], f32, tag="xb", name=f"xb{qp}")
                for nd in range(2):
                    nsl = slice(nd * SQ, (nd + 1) * SQ)
                    nc.vector.scalar_tensor_tensor(
                        out=xbuf[:, nsl],
                        in0=pair[nd],
                        scalar=1.0 / WS,
                        in1=xq_sb[:, qp, nsl],
                        op0=OP.mult,
                        op1=OP.add,
                    )
                ln_qp(qp, xbuf)

    nc.finalize()
    return nc


def _shard_inputs(inputs):
    """Build the 8 per-core input maps from full inputs."""
    import ml_dtypes

    f8 = ml_dtypes.float8_e4m3
    x = np.ascontiguousarray(np.asarray(inputs["hidden_states"], dtype=np.float32))
    mask = np.asarray(inputs["attention_mask"], dtype=np.float32).reshape(B, S)
    W8 = {
        k: np.ascontiguousarray(
            (np.asarray(inputs[k], dtype=np.float32) * WS).astype(f8)
        )
        for k in ("Wq", "Wk", "Wv", "Wo")
    }
    bq = np.asarray(inputs["bq"], dtype=np.float32)
    bv = np.asarray(inputs["bv"], dtype=np.float32)
    bo = np.asarray(inputs["bo"], dtype=np.float32)
    gamma = np.asarray(inputs["ln_gamma"], dtype=np.float32)
    beta = np.asarray(inputs["ln_beta"], dtype=np.float32)
    Wo_f = np.asarray(inputs["Wo"], dtype=np.float32)
    bo_eff = (bv @ Wo_f + bo).astype(np.float32)

    bq_t = np.ascontiguousarray(bq.reshape(DT, P).T)
    gamma_bc = np.broadcast_to(gamma, (P, D))
    beta_bc = np.broadcast_to(beta, (P, D))

    xTb = [np.ascontiguousarray(x[b].T.astype(f8)) for b in range(B)]
    em_row = [np.exp(mask[b]) / WS for b in range(B)]

    in_maps = []
    for c in range(NCORES):
        b, q = c // 4, (c % 4) * SQ
        xT_roll = np.ascontiguousarray(np.roll(xTb[b], -q, axis=1))
        em_t = np.roll(em_row[b], -q).reshape(KS, P).T
        cstv = np.ascontiguousarray(
            np.concatenate([bq_t, em_t, gamma_bc, beta_bc], axis=1).astype(
                np.float32
            )
        )
        in_maps.append(
            {
                "xT8": xT_roll,
                "xq": np.ascontiguousarray(x[b, q : q + SQ, :] + bo_eff),
                "Wq8": W8["Wq"],
                "Wk8": W8["Wk"],
                "Wv8": W8["Wv"],
                "Wo8": W8["Wo"],
                "cst": cstv,
            }
        )
    return in_maps


def run(inputs, trace=False, **kw):
    """Run on hardware; returns (full_output, BassKernelResults)."""
    _ensure_paths()
    from concourse.bass_utils import run_bass_kernel_spmd

    if "nc" not in _CACHE:
        _CACHE["nc"] = build_nc()
    nc = _CACHE["nc"]
    in_maps = _shard_inputs(inputs)
    res = run_bass_kernel_spmd(
        nc, in_maps, core_ids=list(range(NCORES)), trace=trace, **kw
    )
    parts = [res.results[c]["out"] for c in range(NCORES)]
    full = np.empty((B, S, D), dtype=np.float32)
    for c in range(NCORES):
        b, q = c // 4, (c % 4) * SQ
        full[b, q : q + SQ] = parts[c]
    return full, res


def kernel(**inputs):
    out, _ = run(inputs)
    return out


# revision 14
# speedup vs baseline: 1.1698x; 1.1698x over previous
"""BertAttention (B=2,S=2048,D=1024,H=16) on 8 trn2 NeuronCores.

Sharding: data-parallel over B (2 groups of 4 cores); each group's 4 cores
split the 2048 query rows (512 each). Every core computes K^T and V for its
batch in full (redundant within the group; collectives in this runtime cost
15us+ fixed which is worse than the ~40us of redundant PE), its own 512-row
Q slice, attention over all 16 heads, output projection, residual, LayerNorm.
Key columns are host-rotated per core so columns 0:512 of x^T are always the
core's own query block (softmax is key-order invariant) — every core runs an
identical schedule.

v2 structure (one-pass waves, JIT production):
  - All projections run fp8e4 DoubleRow (weights host-scaled by 64; 1/64
    descale folded into the PSUM drains). Biases eliminated exactly: bk
    drops (softmax shift-invariance), bv/bo fold into the host residual,
    bq rides the Q^T drain.
  - Attention is 8 waves x 8 steps; wave w = head pair (2w, 2w+1), step p
    = key tiles 2p,2p+1. The two heads' scores matmuls contract only 64
    partitions and sit on partitions 0:64 / 64:128, so they are issued
    back-to-back and execute CONCURRENTLY in the PE array via row tiling
    (tile_position (0,0) vs (64,0)) — 2x scores throughput.
  - exp is one fused ACT op per (step, head) (scale=1/8, max-free: scores/8
    is in [-3.6,3.6] here), writing fp8 directly. The mask enters as
    exp(mask)/64 folded into V's rows, with exp(mask)/64 in V's 65th column
    so PV row 64 is the softmax denominator /64.
  - PV (fp8 DR) is lagged one step behind scores/exp so the PE never waits
    on the ACT engine; each head's PV accumulates all 16 key tiles in one
    PSUM bank (no SBUF accumulators, no halves).
  - Softmax denominators: reciprocal on DVE, then partition-broadcast via a
    tiny ones-stationary bf16 matmul on the PE (no DRAM bounce).
  - Q/K/V production units are drip-fed into the wave stream on a static
    deadline schedule (emitted >=1 step before first use; PE executes them
    in the slack while ACT runs exp).
  - Tail: all four query-block output projections use all 8 PSUM banks,
    drain+LayerNorm pipelined per block (beta-add on gpsimd for blocks 0-2
    so the DVE chain stays short).
"""

import numpy as np

B, S, D, H = 2, 2048, 1024, 16
HD = D // H  # 64
HD1 = HD + 1
P = 128
NCORES = 8
SQ = S // 4  # 512 query rows per core
DT = D // P  # 8 feature tiles
KS = S // P  # 16 key tiles
WS = 64.0  # host-side weight scale for fp8
EPS = 1e-12
CW = DT + KS + 2 * D  # consts blob width

_CACHE = {}


def _ensure_paths():
    try:
        import concourse  # noqa: F401
    except ImportError:
        import sys

        for p in ("/opt/trn_rl_repo", "/root/.axon_site/_ro/trn_rl_repo"):
            if p not in sys.path:
                sys.path.append(p)
        import concourse  # noqa: F401


def _drip_schedule():
    """Static emission plan for production units.

    Unit kinds: ("q", dt, None), ("k", dt, kc), ("v", kt, nd).
    deadline = first attention step (w*8+p) whose PE instructions read the
    unit's output. Units are levelled into earlier surplus steps, never
    later than deadline-1 (PE executes in issue order; a unit issued at or
    after its consumer would deadlock the in-order queue).
    """
    units = []
    for dt in range(DT):
        units.append(("q", dt, None))
        for kc in range(4):
            units.append(("k", dt, kc))
    for kt in range(KS):
        for nd in range(2):
            units.append(("v", kt, nd))

    def deadline(u):
        kind, a, b = u
        if kind == "q":
            return a * 8
        if kind == "k":
            return a * 8 + 2 * b
        return b * 32 + a // 2  # v: PV flush lags 1 step anyway

    prefix = [("q", 0, None), ("k", 0, 0), ("v", 0, 0), ("v", 1, 0)]
    rest = [u for u in units if u not in prefix]
    rest.sort(key=deadline)
    # latest-fit with cap 1/step (spill backward): keeps the drip JIT so the
    # ACT engine stays exp-dense; wave 0's V deficit is structural.
    plan = [[] for _ in range(64)]
    load = [0] * 64
    for u in rest:
        s = max(0, deadline(u) - 1)
        while s > 0 and load[s] >= 1:
            s -= 1
        plan[s].append(u)
        load[s] += 1
    for s in range(64):
        plan[s].sort(key=deadline)
    return prefix, plan


def build_nc():
    _ensure_paths()
    import concourse.tile as tile
    from concourse import bacc, mybir

    f32 = mybir.dt.float32
    bf16 = mybir.dt.bfloat16
    f8 = mybir.dt.float8e4
    DR = mybir.MatmulPerfMode.DoubleRow
    AF = mybir.ActivationFunctionType
    OP = mybir.AluOpType

    nc = bacc.Bacc()

    # ---- I/O ----
    xT8 = nc.declare_dram_parameter("xT8", [D, S], f8, isOutput=False)
    xq = nc.declare_dram_parameter("xq", [SQ, D], f32, isOutput=False)
    Wq = nc.declare_dram_parameter("Wq8", [D, D], f8, isOutput=False)
    Wk = nc.declare_dram_parameter("Wk8", [D, D], f8, isOutput=False)
    Wv = nc.declare_dram_parameter("Wv8", [D, D], f8, isOutput=False)
    Wo = nc.declare_dram_parameter("Wo8", [D, D], f8, isOutput=False)
    # consts blob: [bq_t | emask_t | gamma_bc | beta_bc]
    cst = nc.declare_dram_parameter("cst", [P, CW], f32, isOutput=False)
    out = nc.declare_dram_parameter("out", [SQ, D], f32, isOutput=True)

    xT_r = xT8.rearrange("(t p) s -> p t s", p=P)  # [128, 8, 2048]
    xq_r = xq.rearrange("(t p) d -> p t d", p=P)  # [128, 4, 1024]
    W_r = {
        "q": Wq.rearrange("(t p) d -> p t d", p=P),
        "k": Wk.rearrange("(t p) d -> p t d", p=P),
        "v": Wv.rearrange("(t p) d -> p t d", p=P),
        "o": Wo.rearrange("(t p) d -> p t d", p=P),
    }
    out_r = out.rearrange("(t p) d -> t p d", p=P)  # [4, 128, 1024]
    # softmax denominators bounce through DRAM for the partition broadcast
    sums_dram = nc.dram_tensor("sums_bounce", [H, SQ], f32)

    def mm(ps, lhsT, rhs, start, stop, perf_mode=None):
        nc.tensor.matmul(ps, lhsT, rhs, start=start, stop=stop, perf_mode=perf_mode)

    prefix, plan = _drip_schedule()

    with tile.TileContext(nc) as tc:
        with (
            tc.tile_pool(name="consts", bufs=1) as consts,
            tc.tile_pool(name="pers", bufs=1) as pers,
            tc.tile_pool(name="wpool", bufs=1) as wpool,
            tc.tile_pool(name="expt", bufs=6) as ex_pool,
            tc.tile_pool(name="sums", bufs=2) as sums_pool,
            tc.tile_pool(name="xbuf", bufs=4) as xb_pool,
            tc.tile_pool(name="stats", bufs=4) as st_pool,
            tc.tile_pool(name="ps_sc", bufs=2, space="PSUM") as ps_sc,
            tc.tile_pool(name="ps_pv", bufs=2, space="PSUM") as ps_pv,
            tc.tile_pool(name="ps_a", bufs=2, space="PSUM") as ps_a,
        ):
            # ---- persistent SBUF ----
            qt_sb = pers.tile([P, DT, SQ], f8)  # Q^T true scale
            kt_sb = pers.tile([P, DT, S], f8)  # K^T true scale
            v_sb = pers.tile([P, KS, H, HD1], f8)  # V*em rows + denom col
            ctxn = pers.tile([P, DT, SQ], f8)  # normalized ctx^T
            cst_sb = consts.tile([P, CW], f32)
            eps_sb = consts.tile([P, 1], f32)
            wq_sb = wpool.tile([P, DT, D], f8, tag="Wq")
            wk_sb = wpool.tile([P, DT, D], f8, tag="Wk")
            wv_sb = wpool.tile([P, DT, D], f8, tag="Wv")
            wo_sb = wpool.tile([P, DT, D], f8, tag="Wo")
            xt8 = pers.tile([P, DT, S], f8)
            xq_sb = pers.tile([P, 4, D], f32)

            bq_sl = cst_sb[:, 0:DT]
            em_sl = cst_sb[:, DT : DT + KS]
            g_sl = cst_sb[:, DT + KS : DT + KS + D]
            be_sl = cst_sb[:, DT + KS + D : DT + KS + 2 * D]

            # ---- DMA wave-up (spread across queues; deadline order) ----
            nc.sync.dma_start(xt8[:, :, 0:SQ], xT_r[:, :, 0:SQ])
            nc.gpsimd.dma_start(cst_sb[:, 0 : DT + KS], cst[:, 0 : DT + KS])
            nc.scalar.dma_start(wq_sb[:, :, 0:P], W_r["q"][:, :, 0:P])
            nc.scalar.dma_start(wk_sb[:, :, 0:P], W_r["k"][:, :, 0:P])
            nc.scalar.dma_start(wv_sb[:, :, 0:SQ], W_r["v"][:, :, 0:SQ])
            nc.sync.dma_start(xt8[:, :, SQ : 2 * SQ], xT_r[:, :, SQ : 2 * SQ])
            nc.sync.dma_start(xt8[:, :, 2 * SQ : 3 * SQ], xT_r[:, :, 2 * SQ : 3 * SQ])
            nc.sync.dma_start(xt8[:, :, 3 * SQ : 4 * SQ], xT_r[:, :, 3 * SQ : 4 * SQ])
            # weight chunks in wave-consumption order (K/Q of wave dt use
            # column chunk dt)
            nc.gpsimd.dma_start(wk_sb[:, :, P : 4 * P], W_r["k"][:, :, P : 4 * P])
            nc.gpsimd.dma_start(wq_sb[:, :, P : 4 * P], W_r["q"][:, :, P : 4 * P])
            nc.gpsimd.dma_start(wk_sb[:, :, 4 * P : D], W_r["k"][:, :, 4 * P : D])
            nc.gpsimd.dma_start(wq_sb[:, :, 4 * P : D], W_r["q"][:, :, 4 * P : D])
            nc.gpsimd.dma_start(wv_sb[:, :, SQ:D], W_r["v"][:, :, SQ:D])
            nc.gpsimd.dma_start(xq_sb[:], xq_r[:])
            nc.gpsimd.dma_start(wo_sb[:], W_r["o"][:])
            nc.gpsimd.dma_start(cst_sb[:, DT + KS :], cst[:, DT + KS :])
            nc.vector.memset(eps_sb[:], EPS)

            # ---- production units ----
            def emit_q(dt):
                ps = ps_a.tile([P, SQ], f32, tag="kv", name=f"qu{dt}")
                for j in range(DT // 2):
                    mm(
                        ps[:],
                        wq_sb[:, 2 * j : 2 * j + 2, dt * P : (dt + 1) * P],
                        xt8[:, 2 * j : 2 * j + 2, 0:SQ],
                        start=(j == 0),
                        stop=(j == DT // 2 - 1),
                        perf_mode=DR,
                    )
                nc.vector.tensor_scalar(
                    out=qt_sb[:, dt, :],
                    in0=ps[:],
                    scalar1=1.0 / WS,
                    scalar2=bq_sl[:, dt : dt + 1],
                    op0=OP.mult,
                    op1=OP.add,
                )

            def emit_k(dt, kc):
                sl = slice(kc * SQ, (kc + 1) * SQ)
                ps = ps_a.tile([P, SQ], f32, tag="kv", name=f"ku{dt}_{kc}")
                for j in range(DT // 2):
                    mm(
                        ps[:],
                        wk_sb[:, 2 * j : 2 * j + 2, dt * P : (dt + 1) * P],
                        xt8[:, 2 * j : 2 * j + 2, sl],
                        start=(j == 0),
                        stop=(j == DT // 2 - 1),
                        perf_mode=DR,
                    )
                nc.vector.tensor_scalar_mul(
                    kt_sb[:, dt, sl], in0=ps[:], scalar1=1.0 / WS
                )

            def emit_v(kt, nd):
                ps = ps_a.tile([P, SQ], f32, tag="kv", name=f"vu{kt}_{nd}")
                for j in range(DT // 2):
                    mm(
                        ps[:],
                        xt8[:, 2 * j : 2 * j + 2, kt * P : (kt + 1) * P],
                        wv_sb[:, 2 * j : 2 * j + 2, nd * SQ : (nd + 1) * SQ],
                        start=(j == 0),
                        stop=(j == DT // 2 - 1),
                        perf_mode=DR,
                    )
                nc.vector.tensor_scalar_mul(
                    v_sb[:, kt, nd * 8 : (nd + 1) * 8, 0:HD],
                    in0=ps[:].rearrange("p (h c) -> p h c", c=HD),
                    scalar1=em_sl[:, kt : kt + 1],
                )
                if nd == 0:
                    # denominator column: exp(mask)/64 per key row. Must land
                    # with the nd=0 unit — wave-0 PV already reads col 64.
                    nc.vector.tensor_copy(
                        v_sb[:, kt, :, HD:HD1],
                        em_sl[:, kt : kt + 1].to_broadcast((P, H, 1)),
                    )

            def emit_unit(u):
                kind, a, b = u
                if kind == "q":
                    emit_q(a)
                elif kind == "k":
                    emit_k(a, b)
                else:
                    emit_v(a, b)

            for u in prefix:
                emit_unit(u)

            # ---- attention: 8 waves x 8 steps, PV lagged one step ----
            pend = {"pv": None}

            def normalize(w, pvts):
                bcss = []
                for hh in range(2):
                    sf = sums_pool.tile(
                        [1, SQ], f32, tag=f"sf{hh}", name=f"sf{w}_{hh}"
                    )
                    nc.vector.tensor_copy(sf[:], pvts[hh][HD:HD1, :])
                    nc.vector.reciprocal_approx_fast(sf[:], sf[:])
                    h = 2 * w + hh
                    nc.sync.dma_start(sums_dram[h : h + 1, :], sf[:])
                    bcs = sums_pool.tile(
                        [HD, SQ], f32, tag=f"bcs{hh}", name=f"bcs{w}_{hh}"
                    )
                    # partition broadcast via DRAM bounce
                    nc.sync.dma_start(
                        bcs[:], sums_dram[h : h + 1, :].to_broadcast((HD, SQ))
                    )
                    bcss.append(bcs)
                for hh in range(2):
                    off = hh * HD
                    nc.vector.scalar_tensor_tensor(
                        out=ctxn[off : off + HD, w, :],
                        in0=pvts[hh][0:HD, :],
                        scalar=1.0 / WS,
                        in1=bcss[hh][:],
                        op0=OP.mult,
                        op1=OP.mult,
                    )

            def flush_pv():
                if pend["pv"] is None:
                    return
                w, p, pvts, exs = pend["pv"]
                pend["pv"] = None
                for hh in range(2):
                    h = 2 * w + hh
                    mm(
                        pvts[hh][:],
                        v_sb[:, 2 * p : 2 * p + 2, h, :],
                        exs[hh][:],
                        start=(p == 0),
                        stop=(p == KS // 2 - 1),
                        perf_mode=DR,
                    )
                if p == KS // 2 - 1:
                    normalize(w, pvts)

            for w in range(8):
                pvps = [
                    ps_pv.tile([HD1, SQ], f32, tag="pv", name=f"pv{w}_{hh}")
                    for hh in range(2)
                ]
                for p in range(8):
                    s = w * 8 + p
                    scs = [
                        ps_sc.tile([P, 2, SQ], f32, tag="sc", name=f"sc{s}_{hh}")
                        for hh in range(2)
                    ]
                    for hh in range(2):
                        off = hh * HD
                        for u in range(2):
                            kt = 2 * p + u
                            mm(
                                scs[hh][:, u, :],
                                kt_sb[off : off + HD, w, kt * P : (kt + 1) * P],
                                qt_sb[off : off + HD, w, :],
                                start=True,
                                stop=True,
                            )
                    flush_pv()
                    exs = []
                    for hh in range(2):
                        ex = ex_pool.tile(
                            [P, 2, SQ], f8, tag="ex", name=f"ex{s}_{hh}"
                        )
                        nc.scalar.activation(
                            ex[:], scs[hh][:], AF.Exp, scale=0.125
                        )
                        exs.append(ex)
                    pend["pv"] = (w, p, pvps, exs)
                    for u2 in plan[s]:
                        emit_unit(u2)

            flush_pv()  # last PV + normalize(7)

            # ---- tail: O proj (all 8 banks) + residual + LayerNorm ----
            def ln_qp(qp, xbuf):
                stats = st_pool.tile([P, 2, 6], f32, tag="st", name=f"st{qp}")
                xv = xbuf[:].rearrange("p (a d) -> p a d", a=2)
                for a in range(2):
                    nc.vector.bn_stats(stats[:, a, :], xv[:, a, :])
                mv = st_pool.tile([P, 2], f32, tag="mv", name=f"mv{qp}")
                nc.vector.bn_aggr(mv[:], stats[:])
                rstd = st_pool.tile([P, 1], f32, tag="rs", name=f"rs{qp}")
                nc.scalar.activation(rstd[:], mv[:, 1:2], AF.Sqrt, bias=eps_sb[:])
                nc.vector.reciprocal(rstd[:], rstd[:])
                nc.vector.tensor_scalar(
                    out=xbuf[:],
                    in0=xbuf[:],
                    scalar1=mv[:, 0:1],
                    scalar2=rstd[:],
                    op0=OP.subtract,
                    op1=OP.mult,
                )
                nc.vector.tensor_mul(xbuf[:], xbuf[:], g_sl[:])
                eng = nc.gpsimd if qp < 3 else nc.vector
                eng.tensor_add(xbuf[:], xbuf[:], be_sl[:])
                nc.sync.dma_start(out_r[qp], xbuf[:])

            for qp in range(4):
                if qp == 0:
                    pair = [
                        ps_a.tile([P, SQ], f32, tag="kv", name=f"oj0_{nd}")
                        for nd in range(2)
                    ]
                elif qp in (1, 2):
                    t = ps_sc.tile([P, 2, SQ], f32, tag="sc", name=f"oj{qp}")
                    pair = [t[:, 0, :], t[:, 1, :]]
                else:
                    pair = [
                        ps_pv.tile([P, SQ], f32, tag="pv", name=f"oj3_{nd}")
                        for nd in range(2)
                    ]
                for nd in range(2):
                    for p4 in range(4):
                        mm(
                            pair[nd],
                            ctxn[:, 2 * p4 : 2 * p4 + 2, qp * P : (qp + 1) * P],
                            wo_sb[:, 2 * p4 : 2 * p4 + 2, nd * SQ : (nd + 1) * SQ],
                            start=(p4 == 0),
                            stop=(p4 == 3),
                            perf_mode=DR,
                        )
                xbuf = xb_pool.tile([P, D], f32, tag="xb", name=f"xb{qp}")
                for nd in range(2):
                    nsl = slice(nd * SQ, (nd + 1) * SQ)
                    nc.vector.scalar_tensor_tensor(
                        out=xbuf[:, nsl],
                        in0=pair[nd],
                        scalar=1.0 / WS,
                        in1=xq_sb[:, qp, nsl],
                        op0=OP.mult,
                        op1=OP.add,
                    )
                ln_qp(qp, xbuf)

    nc.finalize()
    return nc


def _shard_inputs(inputs):
    """Build the 8 per-core input maps from full inputs."""
    import ml_dtypes

    f8 = ml_dtypes.float8_e4m3
    x = np.ascontiguousarray(np.asarray(inputs["hidden_states"], dtype=np.float32))
    mask = np.asarray(inputs["attention_mask"], dtype=np.float32).reshape(B, S)
    W8 = {
        k: np.ascontiguousarray(
            (np.asarray(inputs[k], dtype=np.float32) * WS).astype(f8)
        )
        for k in ("Wq", "Wk", "Wv", "Wo")
    }
    bq = np.asarray(inputs["bq"], dtype=np.float32)
    bv = np.asarray(inputs["bv"], dtype=np.float32)
    bo = np.asarray(inputs["bo"], dtype=np.float32)
    gamma = np.asarray(inputs["ln_gamma"], dtype=np.float32)
    beta = np.asarray(inputs["ln_beta"], dtype=np.float32)
    Wo_f = np.asarray(inputs["Wo"], dtype=np.float32)
    bo_eff = (bv @ Wo_f + bo).astype(np.float32)

    bq_t = np.ascontiguousarray(bq.reshape(DT, P).T)
    gamma_bc = np.broadcast_to(gamma, (P, D))
    beta_bc = np.broadcast_to(beta, (P, D))

    xTb = [np.ascontiguousarray(x[b].T.astype(f8)) for b in range(B)]
    em_row = [np.exp(mask[b]) / WS for b in range(B)]

    in_maps = []
    for c in range(NCORES):
        b, q = c // 4, (c % 4) * SQ
        xT_roll = np.ascontiguousarray(np.roll(xTb[b], -q, axis=1))
        em_t = np.roll(em_row[b], -q).reshape(KS, P).T
        cstv = np.ascontiguousarray(
            np.concatenate([bq_t, em_t, gamma_bc, beta_bc], axis=1).astype(
                np.float32
            )
        )
        in_maps.append(
            {
                "xT8": xT_roll,
                "xq": np.ascontiguousarray(x[b, q : q + SQ, :] + bo_eff),
                "Wq8": W8["Wq"],
                "Wk8": W8["Wk"],
                "Wv8": W8["Wv"],
                "Wo8": W8["Wo"],
                "cst": cstv,
            }
        )
    return in_maps


def run(inputs, trace=False, **kw):
    """Run on hardware; returns (full_output, BassKernelResults)."""
    _ensure_paths()
    from concourse.bass_utils import run_bass_kernel_spmd

    if "nc" not in _CACHE:
        _CACHE["nc"] = build_nc()
    nc = _CACHE["nc"]
    in_maps = _shard_inputs(inputs)
    res = run_bass_kernel_spmd(
        nc, in_maps, core_ids=list(range(NCORES)), trace=trace, **kw
    )
    parts = [res.results[c]["out"] for c in range(NCORES)]
    full = np.empty((B, S, D), dtype=np.float32)
    for c in range(NCORES):
        b, q = c // 4, (c % 4) * SQ
        full[b, q : q + SQ] = parts[c]
    return full, res


def kernel(**inputs):
    out, _ = run(inputs)
    return out
